# revision 61
# baseline (speedup 1.0000x reference)
"""KitNET anomaly-detection ensemble (25 tiny tied-weight autoencoders) on 8 Trainium2 cores.

Strategy (v11, dense-hidden packing, ACT-minimal):
  - Data-parallel over batch: each of the 8 cores processes B/8 = 16384 samples.
  - Host does the feature gather + transpose to feature-major bf16 and lays x
    out tile-major, so each batch tile is ONE DMA with 3KB contiguous lines
    (plus a tiny chunk-3 DMA for AE24's 16 features, landed at partition
    strip 32*(t%4) so the strip-batched AE24 element-wise ops line up).
  - Hidden units are packed DENSELY: slice s of the static enc psum tile E
    holds the 96 hidden rows of chunk s's 8 AEs at rows 12*ai + h; AE24's 12
    hidden rows sit at rows 96..107 of slice 2.  Encode = one K=128, M=96
    matmul per chunk (+ tiny M=12 for AE24 on a free PE cell), so the encode
    sigmoid is ONE [128, 3, 512] ACT instruction (FD 1536).
  - Decode = one K=96, M=128 matmul per chunk into the static psum tile D ->
    ONE dec sigmoid instruction (FD 1536).  AE24's 16-feature decode is
    strip-batched: tile 4j+g writes A24[32g:32g+16] and one [128, 512] ACT
    instruction per 4 tiles sigmoids all four strips (FD 128/tile amortized).
  - ACT is the bottleneck engine (~1 elem/cycle/lane, dtype-independent);
    total sigmoid FD is 3200/tile, near the packing floor.  The loop is
    software-pipelined so ACT alternates enc-sigmoid(t) / dec-sigmoid(t-1)
    back-to-back: iteration t emits prefetch(t+2), head_enc(t), head_dec(t-1),
    tail(t-4).  Weights ride one fused 896-column bf16 blob DMA issued first.
  - PSUM: static E (3 banks) + static D (3) + A24 (1) + S accumulator (1).
  - Tail: err = xt - rec, err^2 on DVE (2x bf16 mode); per-AE sums S[32*
    (t%4) + a, i] via 0/1-matrix matmuls accumulating per strip; AE24's
    contribution lands via 4 strip-matmuls after the batched sigmoid.
  - Phase B (rmse = sqrt(S/16+eps) in bf16, 25-AE sum via a bf16 ones-matmul
    -- bf16 inputs keep the PE in single-pass mode, vs fp32's LOW/HIGH
    two-pass) is emitted for groups 0..6 right after the last sigmoid so it
    overlaps the final tiles' tails; all psum->sbuf copies go on the DVE;
    the ones-matmul outputs reuse D's psum banks.
"""

import sys

for _p in ("/opt/trn_rl_repo", "/opt/pypackages"):
    if _p not in sys.path:
        sys.path.append(_p)

import numpy as np

B = 131072
F = 400          # features
N_AE = 25
KF = 16          # features per AE
H = 12           # hidden per AE
EPS = 1e-6
N_CORES = 8
BC = B // N_CORES    # 16384 samples per core
NB = 512             # batch tile (matmul moving free dim)
NT = BC // NB        # 32 tiles per core

_NC_CACHE = {}


def _build_nc(biases_zero):
    import concourse.tile as tile
    from concourse import bacc, mybir

    f32 = mybir.dt.float32
    bf16 = mybir.dt.bfloat16
    AF = mybir.ActivationFunctionType

    nc = bacc.Bacc()

    xa_d = nc.declare_dram_parameter("xa", [NT * 128, 4, NB], bf16, isOutput=False)
    # one fused bf16 weight blob: wenc 4x96 | wdec 3x128 | g 3x32 | g24 32
    wb_d = nc.declare_dram_parameter("wb", [128, 896], bf16, isOutput=False)
    if not biases_zero:
        bb_d = nc.declare_dram_parameter("bb", [128, 7], f32, isOutput=False)
    y_d = nc.declare_dram_parameter("y", [BC], f32, isOutput=True)

    with tile.TileContext(nc) as tc:
        with (
            tc.tile_pool(name="singles", bufs=1) as singles,
            tc.tile_pool(name="xt", bufs=9) as xt_p,
            tc.tile_pool(name="ht", bufs=4) as ht_p,
            tc.tile_pool(name="rec", bufs=5) as rec_p,
            tc.tile_pool(name="r24", bufs=2) as r24_p,
            tc.tile_pool(name="err", bufs=2) as err_p,
            tc.tile_pool(name="psing", bufs=1, space="PSUM") as psing,
        ):
            wb_sb = singles.tile([128, 896], bf16)
            if not biases_zero:
                bb_sb = singles.tile([128, 7], f32)


            def wenc(c):
                return wb_sb[:, 96 * c:96 * (c + 1)]

            def wdec(c):
                return wb_sb[:, 384 + 128 * c:384 + 128 * (c + 1)]

            def gmat(c):
                return wb_sb[:, 768 + 32 * c:768 + 32 * (c + 1)]

            # static psum: E (enc pre-acts), D (dec pre-acts), A24 (strip-
            # batched AE24 decode pre-acts), sp (per-AE squared-error sums,
            # 4 tiles on partition strips: sall[32*(t%4) + a, t//4, i])
            ew = psing.tile([128, 3, NB], f32)
            dw = psing.tile([128, 3, NB], f32)
            a24 = psing.tile([128, NB], f32)
            sp = psing.tile([128, NB], f32)
            sall = singles.tile([128, NT // 4, NB], f32)

            live = {}

            def prefetch(t, split=False):
                # one DMA per tile: the host packs chunk 3 (AE24's features)
                # into plane 3 rows 32*(t%4).. of the tile-major blob, so the
                # strip-batched AE24 element-wise ops line up (DVE requires
                # operands to share a start partition).  The first tiles are
                # split per plane so the transfers spread across DMA rings
                # and the pipeline fills sooner.
                xt = xt_p.tile([128, 4, NB], bf16, tag="xt")
                if split:
                    g = t % 4
                    nc.sync.dma_start(
                        out=xt[32 * g:32 * g + 16, 3, :],
                        in_=xa_d[t * 128 + 32 * g:t * 128 + 32 * g + 16, 3, :],
                    )
                    for c in range(3):
                        nc.sync.dma_start(
                            out=xt[:, c:c + 1, :],
                            in_=xa_d[t * 128:(t + 1) * 128, c:c + 1, :],
                        )
                else:
                    nc.sync.dma_start(
                        out=xt, in_=xa_d[t * 128:(t + 1) * 128, :, :]
                    )
                live[("x", t)] = xt

            def head_enc(t):
                xt = live[("x", t)]
                # ---- encode: hidden of chunk c's 8 AEs -> E[0:96, c, :];
                # AE24's hidden -> E[96:108, 2, :]
                for c in range(3):
                    nc.tensor.matmul(
                        ew[0:96, c, :],
                        lhsT=wenc(c),
                        rhs=xt[:, c, :],
                        start=True,
                        stop=True,
                    )
                g = t % 4
                nc.tensor.matmul(
                    ew[96:108, 2, :],
                    lhsT=wb_sb[32 * g:32 * g + 16, 288:300],
                    rhs=xt[32 * g:32 * g + 16, 3, :],
                    start=True,
                    stop=True,
                    tile_position=(32 * g, 96),
                )
                ht = ht_p.tile([128, 3, NB], bf16, tag="ht")
                if biases_zero:
                    nc.scalar.activation(out=ht, in_=ew, func=AF.Sigmoid)
                else:
                    for s in range(3):
                        nc.scalar.activation(
                            out=ht[:, s, :], in_=ew[:, s, :],
                            func=AF.Sigmoid, bias=bb_sb[:, s:s + 1],
                        )
                live[("h", t)] = ht

            def head_dec(t):
                g = t % 4
                ht = live.pop(("h", t))
                # ---- decode: rec of chunk c -> D[:, c, :]
                for c in range(3):
                    nc.tensor.matmul(
                        dw[:, c, :],
                        lhsT=wdec(c)[0:96, :],
                        rhs=ht[0:96, c, :],
                        start=True,
                        stop=True,
                    )
                # AE24: strip-batched into a24[32g:32g+16]
                nc.tensor.matmul(
                    a24[32 * g:32 * g + 16, :],
                    lhsT=wb_sb[96:108, 640:656],
                    rhs=ht[96:108, 2, :],
                    start=True,
                    stop=True,
                    tile_position=(96, 32 * g),
                )
                rec = rec_p.tile([128, 3, NB], bf16, tag="rec")
                if biases_zero:
                    nc.scalar.activation(out=rec, in_=dw, func=AF.Sigmoid)
                else:
                    for s in range(3):
                        nc.scalar.activation(
                            out=rec[:, s, :], in_=dw[:, s, :],
                            func=AF.Sigmoid, bias=bb_sb[:, 3 + s:4 + s],
                        )
                live[("r", t)] = rec
                last_group = t // 4 == NT // 4 - 1
                if last_group and g == 1:
                    # split the final group's batched sigmoid so its err/G
                    # chain starts two tiles earlier, shortening the
                    # end-of-kernel critical path
                    r24 = r24_p.tile([128, NB], bf16, tag="r24")
                    if biases_zero:
                        nc.scalar.activation(
                            out=r24[0:64, :], in_=a24[0:64, :], func=AF.Sigmoid
                        )
                    else:
                        nc.scalar.activation(
                            out=r24[0:64, :], in_=a24[0:64, :],
                            func=AF.Sigmoid, bias=bb_sb[0:64, 6:7],
                        )
                    live[("r24", t // 4)] = r24
                elif g == 3:
                    if last_group:
                        r24 = live[("r24", t // 4)]
                        out_ap, in_ap = r24[64:128, :], a24[64:128, :]
                    else:
                        r24 = r24_p.tile([128, NB], bf16, tag="r24")
                        out_ap, in_ap = r24, a24
                        live[("r24", t // 4)] = r24
                    if biases_zero:
                        nc.scalar.activation(out=out_ap, in_=in_ap, func=AF.Sigmoid)
                    else:
                        nc.scalar.activation(
                            out=out_ap, in_=in_ap, func=AF.Sigmoid,
                            bias=bb_sb[:, 6:7] if not last_group
                            else bb_sb[64:128, 6:7],
                        )

            def tail(t):
                g = t % 4
                xt = live[("x", t)]
                rec = live.pop(("r", t))
                # ---- err^2, out-of-place to keep DVE 2x perf mode, then
                # per-AE sums: S[32*(t%4) + a, i] += G^T @ err2 (AE24's
                # contribution is added strip-batched below).  The very last
                # tile is processed per chunk so its G-chain starts as soon
                # as the first chunk's err^2 lands (shorter end chain).
                err = err_p.tile([128, 3, NB], bf16, tag="err")
                if t == NT - 1:
                    for c in range(3):
                        nc.vector.tensor_sub(
                            err[:, c, :], xt[:, c, :], rec[:, c, :]
                        )
                        nc.vector.tensor_mul(
                            rec[:, c, :], err[:, c, :], err[:, c, :]
                        )
                        nc.tensor.matmul(
                            sp[32 * g:32 * (g + 1), :],
                            lhsT=gmat(c),
                            rhs=rec[:, c, :],
                            start=(c == 0),
                            stop=False,
                            tile_position=(0, 32 * g),
                        )
                else:
                    nc.vector.tensor_sub(err, xt[:, 0:3, :], rec)
                    nc.vector.tensor_mul(rec, err, err)
                    for c in range(3):
                        nc.tensor.matmul(
                            sp[32 * g:32 * (g + 1), :],
                            lhsT=gmat(c),
                            rhs=rec[:, c, :],
                            start=(c == 0),
                            stop=False,
                            tile_position=(0, 32 * g),
                        )
                def g24_strips(r24, e24, base, strips):
                    for gg in strips:
                        xtg = live.pop(("x", base + gg))
                        sl = slice(32 * gg, 32 * gg + 16)
                        nc.vector.tensor_sub(
                            e24[sl, :], xtg[sl, 3, :], r24[sl, :]
                        )
                        nc.vector.tensor_mul(r24[sl, :], e24[sl, :], e24[sl, :])
                    for gg in strips:
                        sl = slice(32 * gg, 32 * gg + 16)
                        nc.tensor.matmul(
                            sp[32 * gg:32 * (gg + 1), :],
                            lhsT=wb_sb[sl, 864:896],
                            rhs=r24[sl, :],
                            start=False,
                            stop=True,
                            tile_position=(32 * gg, 32 * gg),
                        )

                last_group = t // 4 == NT // 4 - 1
                if last_group and g == 1:
                    r24 = live[("r24", t // 4)]
                    # allocated from the r24 pool: must stay live until
                    # tail(t+2), which the err pool's ring can't guarantee
                    e24 = r24_p.tile([128, NB], bf16, tag="e24l")
                    live["e24last"] = e24
                    g24_strips(r24, e24, t - 1, (0, 1))
                elif g == 3:
                    r24 = live.pop(("r24", t // 4))
                    if last_group:
                        e24 = live.pop("e24last")
                        g24_strips(r24, e24, t - 3, (2, 3))
                    else:
                        e24 = err_p.tile([128, NB], bf16, tag="e24")
                        g24_strips(r24, e24, t - 3, (0, 1, 2, 3))
                    nc.vector.tensor_copy(out=sall[:, t // 4, :], in_=sp)

            # ---- phase B helpers: rmse = sqrt(S/16 + eps) in bf16, then
            # y = sum over 25 AEs via a bf16 ones-matmul (single PE pass);
            # the matmul outputs borrow D's psum banks (free by then)
            eps_sb = singles.tile([128, 1], f32)
            nc.vector.memset(eps_sb, EPS)
            rmse = singles.tile([128, NT // 4, NB], bf16)
            ones4 = singles.tile([128, 4], bf16)
            nc.gpsimd.memset(ones4, 0.0)
            for g in range(4):
                nc.gpsimd.memset(ones4[32 * g:32 * g + N_AE, g:g + 1], 1.0)
            ybuf = singles.tile([4, NT // 4, NB], f32)

            def phase_b_sqrt(j0, j1):
                nc.scalar.activation(
                    out=rmse[:, j0:j1, :], in_=sall[:, j0:j1, :], func=AF.Sqrt,
                    bias=eps_sb, scale=1.0 / KF,
                )

            # y[b], b = t*NB + i, t = 4j + g  ->  y view [g, j, i]; written
            # out per group so only group 7's tiny DMA sits on the
            # end-of-kernel critical chain
            y_ap = y_d[:].rearrange("(j g i) -> g j i", g=4, i=NB)

            def phase_b_sum(j):
                nc.tensor.matmul(
                    dw[0:4, j % 3, :],
                    lhsT=ones4,
                    rhs=rmse[:, j, :],
                    start=True,
                    stop=True,
                )
                nc.vector.tensor_copy(out=ybuf[:, j, :], in_=dw[0:4, j % 3, :])
                nc.sync.dma_start(
                    out=y_ap[:, j:j + 1, :], in_=ybuf[:, j:j + 1, :]
                )

            LAG = 4
            # tile 0's transfer is the fill-time long pole: issue it before
            # the (smaller) weight blob
            prefetch(0, split=True)
            nc.sync.dma_start(out=wb_sb, in_=wb_d[:, :])
            if not biases_zero:
                nc.sync.dma_start(out=bb_sb, in_=bb_d[:, :])
            prefetch(1, split=True)
            for t in range(NT + LAG):
                if t + 2 < NT:
                    prefetch(t + 2)
                if t < NT:
                    head_enc(t)
                if t >= 1 and t - 1 < NT:
                    head_dec(t - 1)
                if t >= LAG:
                    tail(t - LAG)
                if t == NT:
                    # groups 0..6 are final; overlap their (single-instr)
                    # sqrt + sums with the last tiles' tails (ACT is free
                    # after the last sigmoid, emitted in head_dec(NT-1))
                    phase_b_sqrt(0, 7)
                    for j in range(7):
                        phase_b_sum(j)
            phase_b_sqrt(7, 8)
            phase_b_sum(7)

    nc.compile()
    return nc


def _host_mats(W, hb, vb, idx):
    import ml_dtypes

    bf16 = ml_dtypes.bfloat16
    W = np.asarray(W, np.float32)
    hb = np.asarray(hb, np.float32)
    vb = np.asarray(vb, np.float32)

    wenc = np.zeros((128, 4, 96), np.float32)
    wdec = np.zeros((128, 3, 128), np.float32)
    gmat = np.zeros((128, 3, 32), np.float32)
    g24 = np.zeros((128, 32), np.float32)
    bb = np.zeros((128, 7), np.float32)

    for a in range(24):
        c, ai = a // 8, a % 8
        for k in range(KF):
            for h in range(H):
                wenc[16 * ai + k, c, 12 * ai + h] = W[a, k, h]
                wdec[12 * ai + h, c, 16 * ai + k] = W[a, k, h]
            gmat[16 * ai + k, c, a] = 1.0
            bb[16 * ai + k, 3 + c] = vb[a, k]
        for h in range(H):
            bb[12 * ai + h, c] = hb[a, h]
    # AE24: features at xt[32g:32g+16, 3] (strip per t%4); hidden at E
    # slice 2 rows 96..107; decode strip-batched
    for k in range(KF):
        for h in range(H):
            for g in range(4):
                wenc[32 * g + k, 3, h] = W[24, k, h]
            wdec[96 + h, 2, k] = W[24, k, h]
        for g in range(4):
            g24[32 * g + k, 24] = 1.0
            bb[32 * g + k, 6] = vb[24, k]
    for h in range(H):
        bb[96 + h, 2] = hb[24, h]

    wb = np.concatenate(
        [
            wenc.reshape(128, 384),
            wdec.reshape(128, 384),
            gmat.reshape(128, 96),
            g24,
        ],
        axis=1,
    )
    biases_zero = bool(not (np.any(hb) or np.any(vb)))
    consts = {"wb": np.ascontiguousarray(wb).astype(bf16)}
    if not biases_zero:
        consts["bb"] = np.ascontiguousarray(bb)
    return consts, biases_zero


def _get_nc(biases_zero):
    key = ("nc", biases_zero)
    if key not in _NC_CACHE:
        _NC_CACHE[key] = _build_nc(biases_zero)
    return _NC_CACHE[key]


def _run(x, W, hb, vb, idx, trace=False, **kw):
    import ml_dtypes
    from concourse.bass_utils import run_bass_kernel_spmd

    bf16 = ml_dtypes.bfloat16
    idx = np.asarray(idx)
    # host-side gather: packed column 16a+k = natural feature idx[a, k];
    # then bf16-convert and transpose to feature-major per core, tile-major
    # for chunks 0..2 so each batch tile is one contiguous-line DMA.
    xg = np.asarray(x, np.float32)[:, idx.reshape(-1)].astype(bf16)
    consts, biases_zero = _host_mats(W, hb, vb, idx)
    in_maps = []
    for c in range(N_CORES):
        xt = xg[c * BC:(c + 1) * BC].T  # [400, BC] view
        xa = np.empty((NT, 128, 4, NB), xg.dtype)
        xa[:, :, 0:3, :] = (
            xt[0:384].reshape(3, 128, NT, NB).transpose(2, 1, 0, 3)
        )
        # chunk 3 (AE24's 16 features) for tile t goes to plane 3 rows
        # 32*(t%4)..+16; other plane-3 rows are never read
        xbt = xt[384:400].reshape(16, NT, NB).transpose(1, 0, 2)
        for g in range(4):
            xa[g::4, 32 * g:32 * g + 16, 3, :] = xbt[g::4]
        in_maps.append({"xa": xa.reshape(NT * 128, 4, NB), **consts})
    nc = _get_nc(biases_zero)
    res = run_bass_kernel_spmd(nc, in_maps, list(range(N_CORES)), trace=trace, **kw)
    y = np.concatenate([res.results[c]["y"] for c in range(N_CORES)])
    return y, res


def kernel(x, W, hb, vb, idx):
    y, _ = _run(x, W, hb, vb, idx)
    return y
